# revision 12
# baseline (speedup 1.0000x reference)
"""DAGNN recommender forward pass on 8 Trainium2 NeuronCores (Bass/Tile).

Strategy (v2):
 - Nodes relabeled host-side so each 128-row block has balanced in-edge
   count; 400 blocks, 50 per core.  Within each core, blocks 0..24 form
   the "front" half, 25..49 the "back" half.  The propagation state cur
   is AllGathered per hop as TWO collectives (front table [25600,*] and
   back table [25600,*]) so the second collective overlaps gather work.
 - cur rows are stored as packed hi/lo bf16 pairs (512B per node):
   hi = bf16(v), lo = bf16(v - hi), giving ~f32 precision with bf16
   matmuls (the scatter matmul runs hi and lo against the same one-hot
   selection matrix, accumulating in f32 PSUM).
 - Per hop each dst block runs TWO dma_gathers (front/back source
   window) of its in-edge source rows; indices are int16, padded with
   -1 (negative trailing indices generate no DMA descriptors, so the
   descriptor count matches the true edge count).  Gathers are spread
   over 4 SWDGE queues.
 - Scatter is psum += S.T @ G with S one-hot built on the fly from
   iota/is_equal in bf16.
 - MLPs run feature-major in f32 (same as v1); BatchNorm stats get tiny
   AllReduces; zero-padded fake nodes corrected analytically.
"""

import os
import sys

if "/opt/trn_rl_repo" not in sys.path:
    sys.path.insert(0, "/opt/trn_rl_repo")

import numpy as np


# ---------------------------------------------------------------- config

class Cfg:
    def __init__(self, N=50000, E=800000, K=10, BPC=50, OUT=1000):
        self.N, self.E, self.K, self.BPC, self.OUT = N, E, K, BPC, OUT
        self.H = 128
        self.IN = 128
        self.NCORES = 8
        self.BLK = 128
        self.NBLOCKS = self.NCORES * BPC
        self.NP = self.NBLOCKS * self.BLK
        self.ROWS = BPC * self.BLK              # rows per core
        self.FBPC = BPC // 2                    # front blocks per core
        self.HROWS = self.FBPC * self.BLK       # rows per half per core
        self.WINW = self.NCORES * self.HROWS    # rows per half table
        self.NPADN = self.NP - N
        self.EPS = 1e-5


FULL = Cfg()


# ---------------------------------------------------------------- host prep

def balance_nodes(cfg: Cfg, dst: np.ndarray) -> np.ndarray:
    """perm: old node id (incl. pads) -> new padded row id, balancing
    per-block in-edge counts (LPT greedy)."""
    import heapq
    deg = np.bincount(dst, minlength=cfg.N).astype(np.int64)
    deg_all = np.concatenate([deg, np.zeros(cfg.NP - cfg.N, np.int64)])
    order = np.argsort(-deg_all, kind="stable")
    load = np.zeros(cfg.NBLOCKS, np.int64)
    fill = np.zeros(cfg.NBLOCKS, np.int64)
    perm = np.empty(cfg.NP, np.int64)
    heap = [(0, b) for b in range(cfg.NBLOCKS)]
    heapq.heapify(heap)
    for node in order:
        while True:
            _, b = heapq.heappop(heap)
            if fill[b] < cfg.BLK:
                break
        perm[node] = b * cfg.BLK + fill[b]
        fill[b] += 1
        load[b] += deg_all[node]
        if fill[b] < cfg.BLK:
            heapq.heappush(heap, (load[b], b))
    return perm


def build_edge_structures(cfg: Cfg, edge_index: np.ndarray):
    """Returns (perm, idx_img [8,128,*] i16, dst_img [8,128,*] bf16,
    ACH, BCH)."""
    import ml_dtypes

    src, dst = edge_index[0].astype(np.int64), edge_index[1].astype(np.int64)
    perm = balance_nodes(cfg, dst)
    psrc = perm[src]
    pdst = perm[dst]

    score = psrc // cfg.ROWS
    sbic = (psrc % cfg.ROWS) // cfg.BLK
    sp = psrc % cfg.BLK
    win = (sbic >= cfg.FBPC).astype(np.int64)          # 0=front, 1=back
    grow = score * cfg.HROWS + (sbic % cfg.FBPC) * cfg.BLK + sp

    gb = pdst // cfg.BLK
    rel = pdst % cfg.BLK

    # sort edges by (dst block, window)
    key = gb * 2 + win
    order = np.argsort(key, kind="stable")
    grow, rel, key = grow[order], rel[order], key[order]
    starts = np.searchsorted(key, np.arange(2 * cfg.NBLOCKS + 1))
    counts = np.diff(starts)
    cA = counts[0::2]
    cB = counts[1::2]
    ACH = int(np.ceil(cA.max() / 128))
    BCH = int(np.ceil(cB.max() / 128))
    ASLOTS, BSLOTS = ACH * 128, BCH * 128
    acols, bcols = ASLOTS // 16, BSLOTS // 16

    idx_img = np.full((cfg.NCORES, 16, cfg.BPC * (acols + bcols)), -1, np.int16)
    dst_img = np.full((cfg.NCORES, 128, cfg.BPC * (ACH + BCH)), -1.0,
                      ml_dtypes.bfloat16)
    boff_i = cfg.BPC * acols
    boff_d = cfg.BPC * ACH

    for g in range(cfg.NBLOCKS):
        c, lb = divmod(g, cfg.BPC)
        for w, (slots, nch, cols, ioff, doff) in enumerate([
                (ASLOTS, ACH, acols, lb * acols, lb * ACH),
                (BSLOTS, BCH, bcols, boff_i + lb * bcols, boff_d + lb * BCH)]):
            s, e = starts[2 * g + w], starts[2 * g + w + 1]
            m = e - s
            assert m <= slots, (g, w, m, slots)
            # pad with row 0 (NOT -1: negative idxs crash this runtime's
            # gather ucode; row 0 is a harmless extra fetch, discarded by
            # the -1 entries in the selection matrix)
            iarr = np.zeros(slots, np.int64)
            iarr[:m] = grow[s:e]
            rarr = np.full(slots, -1.0, np.float32)
            rarr[:m] = rel[s:e]
            assert 0 <= iarr[:m].min(initial=0) and \
                iarr[:m].max(initial=0) < cfg.WINW
            idx_img[c, :, ioff:ioff + cols] = \
                iarr.reshape(cols, 16).T.astype(np.int16)
            dst_img[c, :, doff:doff + nch] = \
                rarr.reshape(nch, 128).T.astype(ml_dtypes.bfloat16)

    idx_img = np.tile(idx_img, (1, 8, 1))       # replicate to 128 partitions
    return perm, idx_img, dst_img, ACH, BCH


# ---------------------------------------------------------------- device

def build_nc(cfg: Cfg, ACH: int, BCH: int):
    from concourse import bass, mybir, bacc, tile

    F32 = mybir.dt.float32
    BF16 = mybir.dt.bfloat16
    I16 = mybir.dt.int16
    I32 = mybir.dt.int32
    AF = mybir.ActivationFunctionType
    OP = mybir.AluOpType
    BLK, BPC = cfg.BLK, cfg.BPC
    FBPC, HROWS, WINW = cfg.FBPC, cfg.HROWS, cfg.WINW
    ROWS, K, H, OUT = cfg.ROWS, cfg.K, cfg.H, cfg.OUT
    ASLOTS, BSLOTS = ACH * 128, BCH * 128
    acols, bcols = ASLOTS // 16, BSLOTS // 16
    NCORES = cfg.NCORES
    invN = 1.0 / cfg.N
    OUT_A = min(512, OUT)
    OUT_B = OUT - OUT_A
    MCH = max(ACH, BCH)

    nc = bacc.Bacc("TRN2", target_bir_lowering=False, debug=False,
                   num_devices=NCORES, num_swdge_queues=4)

    x_sh = nc.dram_tensor("x_sh", [ROWS, cfg.IN], F32, kind="ExternalInput")
    idxs = nc.dram_tensor("idxs", [128, BPC * (acols + bcols)], I16,
                          kind="ExternalInput")
    drel = nc.dram_tensor("drel", [128, BPC * (ACH + BCH)], BF16,
                          kind="ExternalInput")
    W1 = nc.dram_tensor("W1", [cfg.IN, H], F32, kind="ExternalInput")
    W2 = nc.dram_tensor("W2", [H, H], F32, kind="ExternalInput")
    W3 = nc.dram_tensor("W3", [H, H // 2], F32, kind="ExternalInput")
    Wout = nc.dram_tensor("Wout", [H // 2, OUT], F32, kind="ExternalInput")
    g1 = nc.dram_tensor("g1", [H, 1], F32, kind="ExternalInput")
    be1 = nc.dram_tensor("be1", [H, 1], F32, kind="ExternalInput")
    g2 = nc.dram_tensor("g2", [H, 1], F32, kind="ExternalInput")
    be2 = nc.dram_tensor("be2", [H, 1], F32, kind="ExternalInput")
    g3 = nc.dram_tensor("g3", [H // 2, 1], F32, kind="ExternalInput")
    be3 = nc.dram_tensor("be3", [H // 2, 1], F32, kind="ExternalInput")
    att = nc.dram_tensor("att", [1, K + 1], F32, kind="ExternalInput")
    bout = nc.dram_tensor("bout", [1, OUT], F32, kind="ExternalInput")
    out = nc.dram_tensor("out", [ROWS, OUT], F32, kind="ExternalOutput")

    rg = [list(range(NCORES))]

    with tile.TileContext(nc) as tc:
        with (
            tc.tile_pool(name="const", bufs=1) as cpool,
            tc.tile_pool(name="resid", bufs=1) as rpool,
            tc.tile_pool(name="dram", bufs=1, space="DRAM") as dpool,
        ):
            # ---------------- persistent DRAM buffers
            # propagation state: hi|lo bf16 packed per node (512B rows).
            # Shared DRAM permits only one writer instruction per tensor, so
            # each hop's AllGather gets its own table.
            curF = [dpool.tile([WINW, 2 * H], BF16, tag=f"curF{i}",
                               name=f"curF{i}", addr_space="Shared")
                    for i in range(K)]
            curB = [dpool.tile([WINW, 2 * H], BF16, tag=f"curB{i}",
                               name=f"curB{i}", addr_space="Shared")
                    for i in range(K)]
            ag_inF = dpool.tile([HROWS, 2 * H], BF16, tag="ag_inF")
            ag_inB = dpool.tile([HROWS, 2 * H], BF16, tag="ag_inB")
            bn_in_d = [dpool.tile([H, 2], F32, tag=f"bni{i}", name=f"bni{i}")
                       for i in range(3)]
            bn_out_d = [dpool.tile([H, 2], F32, tag=f"bno{i}", name=f"bno{i}")
                        for i in range(3)]

            # ---------------- constants / weights to SBUF
            w1sb = cpool.tile([cfg.IN, H], F32)
            nc.sync.dma_start(w1sb[:], W1[:])
            w2sb = cpool.tile([H, H], F32)
            nc.sync.dma_start(w2sb[:], W2[:])
            w3sb = cpool.tile([H, H // 2], F32)
            nc.sync.dma_start(w3sb[:], W3[:])
            wosb = cpool.tile([H // 2, OUT], F32)
            nc.sync.dma_start(wosb[:], Wout[:])
            g1sb = cpool.tile([H, 1], F32); nc.sync.dma_start(g1sb[:], g1[:])
            be1sb = cpool.tile([H, 1], F32); nc.sync.dma_start(be1sb[:], be1[:])
            g2sb = cpool.tile([H, 1], F32); nc.sync.dma_start(g2sb[:], g2[:])
            be2sb = cpool.tile([H, 1], F32); nc.sync.dma_start(be2sb[:], be2[:])
            g3sb = cpool.tile([H // 2, 1], F32); nc.sync.dma_start(g3sb[:], g3[:])
            be3sb = cpool.tile([H // 2, 1], F32); nc.sync.dma_start(be3sb[:], be3[:])
            attsb = cpool.tile([1, K + 1], F32); nc.sync.dma_start(attsb[:], att[:])
            bosb = cpool.tile([1, OUT], F32); nc.sync.dma_start(bosb[:], bout[:])

            idx_sb = cpool.tile([128, BPC * (acols + bcols)], I16)
            nc.sync.dma_start(idx_sb[:], idxs[:])
            dstT = cpool.tile([128, BPC * (ACH + BCH)], BF16)
            nc.sync.dma_start(dstT[:], drel[:])

            identity = cpool.tile([128, 128], F32)
            from concourse.masks import make_identity
            make_identity(nc, identity[:])

            iota_i = cpool.tile([128, MCH * 128], I32)
            nc.gpsimd.iota(iota_i[:].rearrange("p (c d) -> p c d", d=128),
                           pattern=[[0, MCH], [1, 128]], base=0,
                           channel_multiplier=0)
            iotab = cpool.tile([128, MCH * 128], BF16)
            nc.vector.tensor_copy(iotab[:], iota_i[:])

            ones1 = cpool.tile([1, 128], F32)
            nc.vector.memset(ones1[:], 1.0)
            epsc = cpool.tile([128, 1], F32)
            nc.vector.memset(epsc[:], cfg.EPS)

            # softmax(att) -> w[0..K]; wR [128, 2K+1]: w then ratios
            mx = cpool.tile([1, 1], F32)
            nc.vector.tensor_reduce(mx[:], attsb[:], axis=mybir.AxisListType.X,
                                    op=OP.max)
            nmx = cpool.tile([1, 1], F32)
            nc.scalar.mul(nmx[:], mx[:], -1.0)
            ew = cpool.tile([1, K + 1], F32)
            nc.scalar.activation(ew[:], attsb[:], AF.Exp, bias=nmx[:, 0:1],
                                 scale=1.0)
            ssum = cpool.tile([1, 1], F32)
            nc.vector.tensor_reduce(ssum[:], ew[:], axis=mybir.AxisListType.X,
                                    op=OP.add)
            rsum = cpool.tile([1, 1], F32)
            nc.vector.reciprocal(rsum[:], ssum[:])
            wv = cpool.tile([1, K + 1], F32)
            nc.vector.tensor_scalar_mul(wv[:], ew[:], rsum[:, 0:1])
            rw = cpool.tile([1, K + 1], F32)
            nc.vector.reciprocal(rw[:], wv[:])
            wcat = cpool.tile([1, 2 * K + 1], F32)
            nc.vector.tensor_copy(wcat[:, 0:K + 1], wv[:])
            nc.vector.tensor_tensor(out=wcat[:, K + 1:2 * K + 1],
                                    in0=wv[:, 1:K + 1], in1=rw[:, 0:K],
                                    op=OP.mult)
            with tc.tile_pool(name="wps", bufs=1, space="PSUM") as wps:
                wpsum = wps.tile([128, 2 * K + 1], F32, space="PSUM")
                nc.tensor.matmul(out=wpsum[:], lhsT=ones1[:], rhs=wcat[:],
                                 start=True, stop=True)
                wR = cpool.tile([128, 2 * K + 1], F32)
                nc.scalar.copy(wR[:], wpsum[:])

                boutR = cpool.tile([128, OUT], F32)
                bps_a = wps.tile([128, OUT_A], F32, space="PSUM", tag="bps")
                nc.tensor.matmul(out=bps_a[:], lhsT=ones1[:],
                                 rhs=bosb[:, 0:OUT_A], start=True, stop=True)
                nc.scalar.copy(boutR[:, 0:OUT_A], bps_a[:])
                if OUT_B:
                    bps_b = wps.tile([128, OUT_B], F32, space="PSUM", tag="bps")
                    nc.tensor.matmul(out=bps_b[:], lhsT=ones1[:],
                                     rhs=bosb[:, OUT_A:OUT], start=True,
                                     stop=True)
                    nc.scalar.copy(boutR[:, OUT_A:OUT], bps_b[:])

            # ---------------- resident activations
            # stage halves hold hi|lo bf16 per node: block b cols
            # [b*256 : b*256+128]=hi, [+128:+256]=lo
            stageF = rpool.tile([128, FBPC * 2 * H], BF16, tag="stageF")
            stageB = rpool.tile([128, FBPC * 2 * H], BF16, tag="stageB")
            accum = rpool.tile([128, ROWS], F32, tag="accum")

            def stage_slices(b):
                st = stageF if b < FBPC else stageB
                lb = b % FBPC
                hi = st[:, lb * 2 * H: lb * 2 * H + H]
                lo = st[:, lb * 2 * H + H: (lb + 1) * 2 * H]
                return hi, lo

            # ================ input MLP (feature-major) ================
            with (
                tc.tile_pool(name="mlp_a", bufs=1) as apool,
                tc.tile_pool(name="mlp_t", bufs=4) as tpool,
                tc.tile_pool(name="mlp_ps", bufs=3, space="PSUM") as mpps,
                tc.tile_pool(name="stat", bufs=1) as spool,
            ):
                a1 = apool.tile([128, ROWS], F32, tag="a1")
                a2 = apool.tile([128, ROWS], F32, tag="a2")
                scol = spool.tile([128, BPC], F32, tag="scol")
                qcol = spool.tile([128, BPC], F32, tag="qcol")
                bn_sb = [spool.tile([128, 2], F32, tag=f"bnsb{i}",
                                    name=f"bnsb{i}") for i in range(3)]
                bnst = [spool.tile([128, 6], F32, tag=f"bnst{i}",
                                   name=f"bnst{i}") for i in range(3)]

                def bn_stats_finish(i, gsb, besb, parts=128):
                    st = bnst[i]
                    nc.scalar.mul(st[:parts, 0:1], bn_sb[i][:parts, 0:1], invN)
                    nc.scalar.mul(st[:parts, 1:2], bn_sb[i][:parts, 1:2], invN)
                    nc.vector.tensor_tensor(out=st[:parts, 2:3],
                                            in0=st[:parts, 0:1],
                                            in1=st[:parts, 0:1], op=OP.mult)
                    nc.vector.tensor_tensor(out=st[:parts, 2:3],
                                            in0=st[:parts, 1:2],
                                            in1=st[:parts, 2:3], op=OP.subtract)
                    nc.scalar.activation(st[:parts, 3:4], st[:parts, 2:3],
                                         AF.Sqrt, bias=epsc[:parts, 0:1],
                                         scale=1.0)
                    nc.vector.reciprocal(st[:parts, 4:5], st[:parts, 3:4])
                    nc.vector.tensor_tensor(out=st[:parts, 4:5],
                                            in0=st[:parts, 4:5],
                                            in1=gsb[:parts, 0:1], op=OP.mult)
                    nc.vector.tensor_tensor(out=st[:parts, 5:6],
                                            in0=st[:parts, 0:1],
                                            in1=st[:parts, 4:5], op=OP.mult)
                    nc.vector.tensor_tensor(out=st[:parts, 5:6],
                                            in0=besb[:parts, 0:1],
                                            in1=st[:parts, 5:6], op=OP.subtract)

                def bn_allreduce(i, parts=128):
                    nc.sync.dma_start(bn_in_d[i][:parts, :], bn_sb[i][:parts, :])
                    if parts < 128:
                        zf = spool.tile([128 - parts, 2], F32, tag="zfill")
                        nc.vector.memset(zf[:], 0.0)
                        nc.sync.dma_start(bn_in_d[i][parts:, :], zf[:])
                    nc.gpsimd.collective_compute(
                        "AllReduce", OP.add, replica_groups=rg,
                        ins=[bn_in_d[i][:].opt()], outs=[bn_out_d[i][:].opt()])
                    nc.sync.dma_start(bn_sb[i][:parts, :], bn_out_d[i][:parts, :])

                # ---- MLP1: z1T = W1.T @ xT
                for t in range(BPC):
                    xe = tpool.tile([128, 128], F32, tag="xload")
                    nc.sync.dma_start(xe[:], x_sh[t * BLK:(t + 1) * BLK, :])
                    xtp = mpps.tile([128, 128], F32, space="PSUM", tag="xtp")
                    nc.tensor.transpose(xtp[:], xe[:], identity[:])
                    xt = tpool.tile([128, 128], F32, tag="xt")
                    nc.scalar.copy(xt[:], xtp[:])
                    zp = mpps.tile([128, 128], F32, space="PSUM", tag="zp")
                    nc.tensor.matmul(out=zp[:], lhsT=w1sb[:], rhs=xt[:],
                                     start=True, stop=True)
                    tcols = slice(t * BLK, (t + 1) * BLK)
                    nc.scalar.copy(a1[:, tcols], zp[:])
                    nc.vector.tensor_reduce(scol[:, t:t + 1], a1[:, tcols],
                                            axis=mybir.AxisListType.X, op=OP.add)
                    sq = tpool.tile([128, 128], F32, tag="sq")
                    nc.scalar.square(sq[:], a1[:, tcols])
                    nc.vector.tensor_reduce(qcol[:, t:t + 1], sq[:],
                                            axis=mybir.AxisListType.X, op=OP.add)
                nc.vector.tensor_reduce(bn_sb[0][:, 0:1], scol[:],
                                        axis=mybir.AxisListType.X, op=OP.add)
                nc.vector.tensor_reduce(bn_sb[0][:, 1:2], qcol[:],
                                        axis=mybir.AxisListType.X, op=OP.add)
                bn_allreduce(0)
                bn_stats_finish(0, g1sb, be1sb)
                sc1 = bnst[0][:, 4:5]
                sh1 = bnst[0][:, 5:6]
                for t in range(BPC):
                    tcols = slice(t * BLK, (t + 1) * BLK)
                    nc.scalar.activation(a1[:, tcols], a1[:, tcols], AF.Relu,
                                         bias=sh1, scale=sc1)

                hpad1 = spool.tile([128, 1], F32, tag="hpad1")
                nc.scalar.activation(hpad1[:], sh1, AF.Relu)
                zpad2 = spool.tile([128, 1], F32, tag="zpad2")
                zp2p = mpps.tile([128, 1], F32, space="PSUM", tag="zp")
                nc.tensor.matmul(out=zp2p[:], lhsT=w2sb[:], rhs=hpad1[:],
                                 start=True, stop=True)
                nc.scalar.copy(zpad2[:], zp2p[:])

                # ---- MLP2: z2T = W2.T @ h1T
                for t in range(BPC):
                    tcols = slice(t * BLK, (t + 1) * BLK)
                    zp = mpps.tile([128, 128], F32, space="PSUM", tag="zp")
                    nc.tensor.matmul(out=zp[:], lhsT=w2sb[:], rhs=a1[:, tcols],
                                     start=True, stop=True)
                    nc.scalar.copy(a2[:, tcols], zp[:])
                    nc.vector.tensor_reduce(scol[:, t:t + 1], a2[:, tcols],
                                            axis=mybir.AxisListType.X, op=OP.add)
                    sq = tpool.tile([128, 128], F32, tag="sq")
                    nc.scalar.square(sq[:], a2[:, tcols])
                    nc.vector.tensor_reduce(qcol[:, t:t + 1], sq[:],
                                            axis=mybir.AxisListType.X, op=OP.add)
                nc.vector.tensor_reduce(bn_sb[1][:, 0:1], scol[:],
                                        axis=mybir.AxisListType.X, op=OP.add)
                nc.vector.tensor_reduce(bn_sb[1][:, 1:2], qcol[:],
                                        axis=mybir.AxisListType.X, op=OP.add)
                bn_allreduce(1)
                corr = spool.tile([128, 2], F32, tag="corr")
                nc.scalar.mul(corr[:, 0:1], zpad2[:], -float(cfg.NPADN))
                sqz = spool.tile([128, 1], F32, tag="sqz")
                nc.scalar.square(sqz[:], zpad2[:])
                nc.scalar.mul(corr[:, 1:2], sqz[:], -float(cfg.NPADN))
                nc.vector.tensor_tensor(out=bn_sb[1][:], in0=bn_sb[1][:],
                                        in1=corr[:], op=OP.add)
                bn_stats_finish(1, g2sb, be2sb)
                sc2 = bnst[1][:, 4:5]
                sh2 = bnst[1][:, 5:6]
                for t in range(BPC):
                    tcols = slice(t * BLK, (t + 1) * BLK)
                    nc.scalar.activation(a2[:, tcols], a2[:, tcols], AF.Relu,
                                         bias=sh2, scale=sc2)
                    nc.vector.tensor_tensor(out=a2[:, tcols], in0=a2[:, tcols],
                                            in1=a1[:, tcols], op=OP.add)

                hpad2 = spool.tile([128, 1], F32, tag="hpad2")
                nc.scalar.activation(hpad2[:], zpad2[:], AF.Relu, bias=sh2,
                                     scale=sc2)
                nc.vector.tensor_tensor(out=hpad2[:], in0=hpad2[:],
                                        in1=hpad1[:], op=OP.add)
                w0hpad2 = spool.tile([128, 1], F32, tag="w0hpad2")
                nc.scalar.activation(w0hpad2[:], hpad2[:], AF.Copy, bias=0.0,
                                     scale=wR[:, 0:1])
                zpad3 = spool.tile([64, 1], F32, tag="zpad3")
                zp3p = mpps.tile([64, 1], F32, space="PSUM", tag="zp")
                nc.tensor.matmul(out=zp3p[:], lhsT=w3sb[:], rhs=w0hpad2[:],
                                 start=True, stop=True)
                nc.scalar.copy(zpad3[:], zp3p[:])
                zpad3_keep = rpool.tile([64, 2], F32, tag="zpad3k")
                nc.scalar.mul(zpad3_keep[:, 0:1], zpad3[:], -float(cfg.NPADN))
                sq3 = spool.tile([64, 1], F32, tag="sq3")
                nc.scalar.square(sq3[:], zpad3[:])
                nc.scalar.mul(zpad3_keep[:, 1:2], sq3[:], -float(cfg.NPADN))

                # stage = h2 node-major hi/lo; accum = w0 * h2
                for t in range(BPC):
                    tcols = slice(t * BLK, (t + 1) * BLK)
                    htp = mpps.tile([128, 128], F32, space="PSUM", tag="xtp")
                    nc.tensor.transpose(htp[:], a2[:, tcols], identity[:])
                    hi, lo = stage_slices(t)
                    nc.scalar.copy(hi, htp[:])
                    hif = tpool.tile([128, 128], F32, tag="hif")
                    nc.vector.tensor_copy(hif[:], hi)
                    nc.vector.tensor_tensor(out=lo, in0=htp[:], in1=hif[:],
                                            op=OP.subtract)
                    nc.scalar.activation(accum[:, tcols], htp[:],
                                         AF.Copy, bias=0.0, scale=wR[:, 0:1])

            # first AllGather pair
            nc.sync.dma_start(
                ag_inF[:].rearrange("(b p) f -> p b f", p=BLK),
                stageF[:].rearrange("p (b f) -> p b f", f=2 * H))
            nc.gpsimd.collective_compute(
                "AllGather", mybir.AluOpType.bypass, replica_groups=rg,
                ins=[ag_inF[:].opt()], outs=[curF[0][:, :].opt()])
            nc.sync.dma_start(
                ag_inB[:].rearrange("(b p) f -> p b f", p=BLK),
                stageB[:].rearrange("p (b f) -> p b f", f=2 * H))
            nc.gpsimd.collective_compute(
                "AllGather", mybir.AluOpType.bypass, replica_groups=rg,
                ins=[ag_inB[:].opt()], outs=[curB[0][:, :].opt()])

            # ================ K propagation hops ================
            with (
                tc.tile_pool(name="gatA", bufs=6) as gApool,
                tc.tile_pool(name="gatB", bufs=6) as gBpool,
                tc.tile_pool(name="spool", bufs=4) as sbpool,
                tc.tile_pool(name="tmpp", bufs=4) as tpool2,
                tc.tile_pool(name="hps", bufs=4, space="PSUM") as hpps,
            ):
                # pre-zero the gather rings (stale data is multiplied by the
                # -1-slot zero rows of S; NaN/Inf garbage would poison 0*x)
                for _ in range(6):
                    gA0 = gApool.tile([128, ACH * 2 * H], BF16, tag="gA")
                    nc.vector.memset(gA0[:], 0.0)
                    gB0 = gBpool.tile([128, BCH * 2 * H], BF16, tag="gB")
                    nc.vector.memset(gB0[:], 0.0)

                qrr = 0

                def gather(pool, tag, nch, slots, cols, rdtab, ioff):
                    nonlocal qrr
                    gt = pool.tile([128, nch * 2 * H], BF16, tag=tag)
                    nc.gpsimd.dma_gather(
                        gt[:].rearrange("p (c f) -> p c f", f=2 * H),
                        rdtab[:, :],
                        idx_sb[:, ioff:ioff + cols],
                        slots, slots, 2 * H, single_packet=False,
                        queue_num=qrr % 4)
                    qrr += 1
                    return gt

                def block_compute(b, gA, gB, rcol):
                    ps = hpps.tile([128, 128], F32, space="PSUM", tag="hps")
                    sA = sbpool.tile([128, ACH * 128], BF16, tag="sA")
                    nc.vector.tensor_tensor(
                        out=sA[:].rearrange("p (c d) -> p c d", d=128),
                        in0=iotab[:, 0:ACH * 128]
                            .rearrange("p (c d) -> p c d", d=128),
                        in1=dstT[:, b * ACH:(b + 1) * ACH]
                            .rearrange("p (c d) -> p c d", d=1)
                            .to_broadcast([128, ACH, 128]),
                        op=OP.is_equal)
                    sB = sbpool.tile([128, BCH * 128], BF16, tag="sB")
                    boff_d = BPC * ACH
                    nc.vector.tensor_tensor(
                        out=sB[:].rearrange("p (c d) -> p c d", d=128),
                        in0=iotab[:, 0:BCH * 128]
                            .rearrange("p (c d) -> p c d", d=128),
                        in1=dstT[:, boff_d + b * BCH: boff_d + (b + 1) * BCH]
                            .rearrange("p (c d) -> p c d", d=1)
                            .to_broadcast([128, BCH, 128]),
                        op=OP.is_equal)
                    for k in range(ACH):
                        for h in range(2):          # hi, lo
                            nc.tensor.matmul(
                                out=ps[:],
                                lhsT=sA[:, k * 128:(k + 1) * 128],
                                rhs=gA[:, k * 2 * H + h * H:
                                       k * 2 * H + (h + 1) * H],
                                start=(k == 0 and h == 0), stop=False)
                    for k in range(BCH):
                        for h in range(2):
                            nc.tensor.matmul(
                                out=ps[:],
                                lhsT=sB[:, k * 128:(k + 1) * 128],
                                rhs=gB[:, k * 2 * H + h * H:
                                       k * 2 * H + (h + 1) * H],
                                start=False,
                                stop=(k == BCH - 1 and h == 1))
                    # stage hi/lo + accum
                    tcols = slice(b * BLK, (b + 1) * BLK)
                    hi, lo = stage_slices(b)
                    tmp = tpool2.tile([128, 128], F32, tag="tmp")
                    nc.scalar.activation(tmp[:], ps[:], AF.Copy, bias=0.0,
                                         scale=rcol)
                    nc.vector.tensor_tensor(out=accum[:, tcols],
                                            in0=accum[:, tcols],
                                            in1=tmp[:], op=OP.add)
                    nc.scalar.copy(hi, tmp[:])
                    hif = tpool2.tile([128, 128], F32, tag="hif2")
                    nc.vector.tensor_copy(hif[:], hi)
                    nc.vector.tensor_tensor(out=lo, in0=tmp[:], in1=hif[:],
                                            op=OP.subtract)

                for i in range(1, K + 1):
                    rdF = curF[i - 1]
                    rdB = curB[i - 1]
                    rcol = wR[:, K + i: K + i + 1]
                    for b in range(BPC):
                        gA = gather(gApool, "gA", ACH, ASLOTS, acols, rdF,
                                    b * acols)
                        gB = gather(gBpool, "gB", BCH, BSLOTS, bcols, rdB,
                                    BPC * acols + b * bcols)
                        block_compute(b, gA, gB, rcol)
                        if b == FBPC - 1 and i < K:
                            nc.sync.dma_start(
                                ag_inF[:].rearrange("(b p) f -> p b f", p=BLK),
                                stageF[:].rearrange("p (b f) -> p b f",
                                                    f=2 * H))
                            nc.gpsimd.collective_compute(
                                "AllGather", mybir.AluOpType.bypass,
                                replica_groups=rg,
                                ins=[ag_inF[:].opt()],
                                outs=[curF[i][:, :].opt()])
                    if i < K:
                        nc.sync.dma_start(
                            ag_inB[:].rearrange("(b p) f -> p b f", p=BLK),
                            stageB[:].rearrange("p (b f) -> p b f", f=2 * H))
                        nc.gpsimd.collective_compute(
                            "AllGather", mybir.AluOpType.bypass,
                            replica_groups=rg,
                            ins=[ag_inB[:].opt()],
                            outs=[curB[i][:, :].opt()])

            # ================ output MLP ================
            with (
                tc.tile_pool(name="tail_a", bufs=1) as tapool,
                tc.tile_pool(name="tail_t", bufs=4) as ttpool,
                tc.tile_pool(name="tail_ps", bufs=2, space="PSUM") as tpps,
                tc.tile_pool(name="tstat", bufs=1) as tspool,
            ):
                a3 = tapool.tile([64, ROWS], F32, tag="a3")
                scol3 = tspool.tile([64, BPC], F32, tag="scol3")
                qcol3 = tspool.tile([64, BPC], F32, tag="qcol3")
                bn3_sb = tspool.tile([128, 2], F32, tag="bn3sb")
                bn3st = tspool.tile([64, 6], F32, tag="bn3st")

                for t in range(BPC):
                    tcols = slice(t * BLK, (t + 1) * BLK)
                    otp = tpps.tile([128, 128], F32, space="PSUM", tag="otp")
                    nc.tensor.transpose(otp[:], accum[:, tcols], identity[:])
                    ot = ttpool.tile([128, 128], F32, tag="ot")
                    nc.scalar.copy(ot[:], otp[:])
                    zp = tpps.tile([64, 128], F32, space="PSUM", tag="zp3")
                    nc.tensor.matmul(out=zp[:], lhsT=w3sb[:], rhs=ot[:],
                                     start=True, stop=True)
                    nc.scalar.copy(a3[:, tcols], zp[:])
                    nc.vector.tensor_reduce(scol3[:, t:t + 1], a3[:, tcols],
                                            axis=mybir.AxisListType.X, op=OP.add)
                    sq = ttpool.tile([64, 128], F32, tag="sq3t")
                    nc.scalar.square(sq[:], a3[:, tcols])
                    nc.vector.tensor_reduce(qcol3[:, t:t + 1], sq[:],
                                            axis=mybir.AxisListType.X, op=OP.add)
                nc.vector.tensor_reduce(bn3_sb[:64, 0:1], scol3[:],
                                        axis=mybir.AxisListType.X, op=OP.add)
                nc.vector.tensor_reduce(bn3_sb[:64, 1:2], qcol3[:],
                                        axis=mybir.AxisListType.X, op=OP.add)
                nc.sync.dma_start(bn_in_d[2][:64, :], bn3_sb[:64, :])
                zf = tspool.tile([64, 2], F32, tag="zf3")
                nc.vector.memset(zf[:], 0.0)
                nc.sync.dma_start(bn_in_d[2][64:, :], zf[:])
                nc.gpsimd.collective_compute(
                    "AllReduce", OP.add, replica_groups=rg,
                    ins=[bn_in_d[2][:].opt()], outs=[bn_out_d[2][:].opt()])
                nc.sync.dma_start(bn3_sb[:64, :], bn_out_d[2][:64, :])
                nc.vector.tensor_tensor(out=bn3_sb[:64, :], in0=bn3_sb[:64, :],
                                        in1=zpad3_keep[:], op=OP.add)
                st = bn3st
                nc.scalar.mul(st[:, 0:1], bn3_sb[:64, 0:1], invN)
                nc.scalar.mul(st[:, 1:2], bn3_sb[:64, 1:2], invN)
                nc.vector.tensor_tensor(out=st[:, 2:3], in0=st[:, 0:1],
                                        in1=st[:, 0:1], op=OP.mult)
                nc.vector.tensor_tensor(out=st[:, 2:3], in0=st[:, 1:2],
                                        in1=st[:, 2:3], op=OP.subtract)
                nc.scalar.activation(st[:, 3:4], st[:, 2:3], AF.Sqrt,
                                     bias=epsc[:64, 0:1], scale=1.0)
                nc.vector.reciprocal(st[:, 4:5], st[:, 3:4])
                nc.vector.tensor_tensor(out=st[:, 4:5], in0=st[:, 4:5],
                                        in1=g3sb[:, 0:1], op=OP.mult)
                nc.vector.tensor_tensor(out=st[:, 5:6], in0=st[:, 0:1],
                                        in1=st[:, 4:5], op=OP.mult)
                nc.vector.tensor_tensor(out=st[:, 5:6], in0=be3sb[:, 0:1],
                                        in1=st[:, 5:6], op=OP.subtract)

                for t in range(BPC):
                    tcols = slice(t * BLK, (t + 1) * BLK)
                    nc.scalar.activation(a3[:, tcols], a3[:, tcols], AF.Relu,
                                         bias=st[:, 5:6], scale=st[:, 4:5])
                    po_a = tpps.tile([128, OUT_A], F32, space="PSUM", tag="poa")
                    nc.tensor.matmul(out=po_a[:], lhsT=a3[:, tcols],
                                     rhs=wosb[:, 0:OUT_A], start=True, stop=True)
                    ost = ttpool.tile([128, OUT], F32, tag="ost")
                    nc.vector.tensor_tensor(out=ost[:, 0:OUT_A], in0=po_a[:],
                                            in1=boutR[:, 0:OUT_A], op=OP.add)
                    if OUT_B:
                        po_b = tpps.tile([128, OUT_B], F32, space="PSUM",
                                         tag="pob")
                        nc.tensor.matmul(out=po_b[:], lhsT=a3[:, tcols],
                                         rhs=wosb[:, OUT_A:OUT], start=True,
                                         stop=True)
                        nc.vector.tensor_tensor(out=ost[:, OUT_A:OUT],
                                                in0=po_b[:],
                                                in1=boutR[:, OUT_A:OUT],
                                                op=OP.add)
                    nc.sync.dma_start(out[t * BLK:(t + 1) * BLK, :], ost[:])

    nc.compile()
    return nc


# ---------------------------------------------------------------- runner

_CACHE = {}


def run(inputs: dict, cfg: Cfg, trace: bool = False):
    from concourse.bass_utils import run_bass_kernel_spmd

    edge_index = np.asarray(inputs["edge_index"])
    perm, idx_img, dst_img, ACH, BCH = build_edge_structures(cfg, edge_index)

    x = np.asarray(inputs["x"], np.float32)
    xp = np.zeros((cfg.NP, cfg.IN), np.float32)
    xp[perm[:cfg.N]] = x

    def col(v, parts):
        return np.asarray(v, np.float32).reshape(parts, 1)

    in_maps = []
    for c in range(cfg.NCORES):
        in_maps.append({
            "x_sh": xp[c * cfg.ROWS:(c + 1) * cfg.ROWS],
            "idxs": idx_img[c],
            "drel": dst_img[c],
            "W1": np.asarray(inputs["W1"], np.float32),
            "W2": np.asarray(inputs["W2"], np.float32),
            "W3": np.asarray(inputs["W3"], np.float32),
            "Wout": np.asarray(inputs["Wout"], np.float32),
            "g1": col(inputs["g1"], 128), "be1": col(inputs["be1"], 128),
            "g2": col(inputs["g2"], 128), "be2": col(inputs["be2"], 128),
            "g3": col(inputs["g3"], 64), "be3": col(inputs["be3"], 64),
            "att": np.asarray(inputs["att"], np.float32).reshape(1, -1),
            "bout": np.asarray(inputs["bout"], np.float32).reshape(1, -1),
        })

    key = (cfg.N, cfg.E, cfg.K, cfg.BPC, cfg.OUT, ACH, BCH)
    if key not in _CACHE:
        _CACHE[key] = build_nc(cfg, ACH, BCH)
    nc = _CACHE[key]

    res = run_bass_kernel_spmd(nc, in_maps, core_ids=list(range(cfg.NCORES)),
                               trace=trace)
    outp = np.concatenate([res.results[c]["out"] for c in range(cfg.NCORES)], 0)
    outf = outp[perm[:cfg.N]]
    return outf.astype(np.float32), res


def kernel(**inputs) -> np.ndarray:
    out, _ = run(inputs, FULL)
    return out
